# revision 1
# baseline (speedup 1.0000x reference)
"""CapsuleLayer dynamic-routing kernel for 8 Trainium2 NeuronCores.

Algorithm (validated vs reference in numpy):
  priors P[c,b,n,o] = sum_i x[b,n,i] W[c,n,i,o]; logits are constant along o,
  so routing state is L[c,b,n]. Per routing iteration:
    probs = exp(L)/denom       (softmax over n; no max-subtraction: |L| < ~30)
    s[c,b,o] = sum_n probs*P = (1/denom) sum_{(n,i)} (x*exp(L)) W   <- matmul
    v = squash(s) = s_u * g,  g = nrm/((1+nrm)*sqrt(nrm+eps))/denom
    a[c,b,n] = sum_o P*v = sum_i x * (W^T v)       <- matmul + blockdiag reduce
    L += a
  Sharding: N=1152 split 8 ways (144 route nodes/core); one 174KB AllReduce of
  s-partials + softmax denominators per iteration. Every core computes the
  identical full output; core 0's is returned.
"""

import sys

sys.path.insert(0, "/opt/trn_rl_repo")

import numpy as np
import ml_dtypes

import concourse.bass as bass
import concourse.bacc as bacc
import concourse.mybir as mybir
from concourse import bass_utils
from concourse.tile import TileContext

BF16 = mybir.dt.bfloat16
F32 = mybir.dt.float32
F16 = mybir.dt.float16
AF = mybir.ActivationFunctionType
ALU = mybir.AluOpType

B, N, CI, C, CO = 256, 1152, 8, 10, 16
NCORES = 8
NLOC = N // NCORES          # 144 route nodes per core
K = NLOC * CI               # 1152 local contraction length (n,i)
NCH = K // 128              # 9 partition chunks of (n,i)
NFULL = 128 // CI           # 16 n per chunk
EPS = 1e-8
NITER = 3
CB = C * B                  # 2560


def _build_blockdiag() -> np.ndarray:
    """a-reduce lhsT constants: cols 0..1023 hold 8 [128,128] blocks (chunk j
    maps (n16,i8) row q -> out partition 16j + q//8); cols 1024..1039 hold the
    9th chunk's [128,16] block (out partition q//8)."""
    blk = np.zeros((128, 8 * 128 + 16), np.float32)
    for j in range(8):
        for q in range(128):
            blk[q, 128 * j + 16 * j + q // CI] = 1.0
    for q in range(128):
        blk[q, 1024 + q // CI] = 1.0
    return blk.astype(np.float16)


def _bcast_ap(ap, dim_idx, count):
    """Insert a stride-0 (broadcast) dim into an AP at position dim_idx."""
    dims = [list(d) for d in ap.ap]
    dims.insert(dim_idx, [0, count])
    return bass.AP(tensor=ap.tensor, offset=ap.offset, ap=dims)


def _reshaped_ap(ap, dims):
    return bass.AP(tensor=ap.tensor, offset=ap.offset, ap=[list(d) for d in dims])


def build_kernel():
    nc = bacc.Bacc("TRN2", target_bir_lowering=False, debug=False,
                   num_devices=NCORES)
    xT_d = nc.dram_tensor("xT", [K, B], BF16, kind="ExternalInput")
    xTf_d = nc.dram_tensor("xTf", [K, B], F32, kind="ExternalInput")
    w1_d = nc.dram_tensor("w1", [C, K, CO], BF16, kind="ExternalInput")
    w2_d = nc.dram_tensor("w2", [C, CO, K], F16, kind="ExternalInput")
    blk_d = nc.dram_tensor("blk", [128, 1040], F16, kind="ExternalInput")
    vout_d = nc.dram_tensor("vout", [CO, CB], F32, kind="ExternalOutput")

    with TileContext(nc) as tc:
        _emit(tc, xT_d.ap(), xTf_d.ap(), w1_d.ap(), w2_d.ap(), blk_d.ap(), vout_d.ap())
    nc.compile()
    return nc


def _emit(tc, xT_d, xTf_d, w1_d, w2_d, blk_d, vout_d):
    from contextlib import ExitStack
    with ExitStack() as ctx:
        _emit_body(ctx, tc, xT_d, xTf_d, w1_d, w2_d, blk_d, vout_d)


def _emit_body(ctx, tc, xT_d, xTf_d, w1_d, w2_d, blk_d, vout_d):
    nc = tc.nc
    state = ctx.enter_context(tc.tile_pool(name="state", bufs=1))
    erep_p = ctx.enter_context(tc.tile_pool(name="erep", bufs=2))
    z_p = ctx.enter_context(tc.tile_pool(name="zp", bufs=2))
    gtmp_p = ctx.enter_context(tc.tile_pool(name="gtmp", bufs=2))
    dram = ctx.enter_context(tc.tile_pool(name="dram", bufs=2, space="DRAM"))
    ups_p = ctx.enter_context(tc.tile_pool(name="ups", bufs=2, space="PSUM"))
    acc_p = ctx.enter_context(tc.tile_pool(name="acc", bufs=3, space="PSUM"))
    tiny_p = ctx.enter_context(tc.tile_pool(name="tinyps", bufs=1, space="PSUM"))

    # ---- persistent SBUF state ----
    xT = state.tile([128, NCH * B], BF16)        # [(n,i) chunk-part, (j, b)]
    xTf = state.tile([128, NCH * B], F32)        # fp32 copy for agreement
    w1 = state.tile([128, C * NCH * CO], BF16)   # s-matmul lhsT blocks
    w2 = state.tile([16, C * K], F16)            # U-matmul lhsT blocks
    blk = state.tile([128, 1040], F16)           # a-reduce lhsT blocks
    ones128 = state.tile([128, 1], BF16)
    ones16f = state.tile([16, 1], F32)
    L = state.tile([128, CB], F32)               # logits, partition = local n
    L9 = state.tile([16, CB], F32)               # local n in [128,144)
    expL = state.tile([128, CB], BF16)
    expL9 = state.tile([16, CB], BF16)
    y_all = state.tile([128, C * NCH * B], BF16)  # y = x*expL per c
    s_u = state.tile([16, CB], F32)              # AllReduced s_unnorm [o,(c,b)]
    s2 = state.tile([16, CB], BF16)
    s_part = state.tile([16, CB], F32)
    ones16b = state.tile([16, 1], BF16)
    v_f = state.tile([16, CB], F32)
    g_rep = state.tile([16, CB], F32)
    vb = state.tile([16, CB], F16)
    den_g = state.tile([128, 20], F32)
    ssq_g = state.tile([128, 20], F32)

    # ---- load inputs / init state ----
    for j in range(NCH):
        nc.sync.dma_start(out=xT[:, j * B:(j + 1) * B],
                          in_=xT_d[j * 128:(j + 1) * 128, :])
        nc.sync.dma_start(out=xTf[:, j * B:(j + 1) * B],
                          in_=xTf_d[j * 128:(j + 1) * 128, :])
    for c in range(C):
        # w1[c] chunk j of 128 (n,i)-rows -> w1 cols (c*NCH+j)*CO .. +CO
        src = w1_d[c].rearrange("(j p) o -> p j o", j=NCH)
        dst = w1[:, c * NCH * CO:(c + 1) * NCH * CO].rearrange(
            "p (j o) -> p j o", j=NCH)
        nc.sync.dma_start(out=dst, in_=src)
    nc.sync.dma_start(out=w2[:].rearrange("p (c k) -> p c k", c=C),
                      in_=w2_d.rearrange("c o k -> o c k"))
    nc.sync.dma_start(out=blk[:], in_=blk_d[:, :])
    eps128 = state.tile([128, 1], F32)
    nc.vector.memset(eps128[:], EPS)
    nc.vector.memset(ones128[:], 1.0)
    nc.vector.memset(ones16f[:], 1.0)
    nc.vector.memset(ones16b[:], 1.0)
    nc.vector.memset(L[:], 0.0)
    nc.vector.memset(L9[:], 0.0)
    nc.vector.memset(expL[:], 1.0)   # exp(0)
    nc.vector.memset(expL9[:], 1.0)

    HC = C // 2          # capsules per half-collective
    HB = HC * B          # 1280
    # blob rows: [0..16*HC) = s partials [(c,o), b]; [16*HC..16*HC+HC) = denom
    RB = 16 * HC + HC    # 85

    def s_matmuls(c, it, blob):
        s_ps = acc_p.tile([16, B], F32, tag="acc", name=f"s_ps_{it}_{c}")
        for j in range(NCH):
            rhs = (xT[:, j * B:(j + 1) * B] if it == 0 else
                   y_all[:, (c * NCH + j) * B:(c * NCH + j + 1) * B])
            lo = (c * NCH + j) * CO
            nc.tensor.matmul(s_ps[:], w1[:, lo:lo + CO], rhs,
                             start=(j == 0), stop=(j == NCH - 1))
        nc.scalar.copy(s_part[:, c * B:(c + 1) * B], s_ps[:])
        ch = c % HC
        nc.sync.dma_start(out=blob[16 * ch:16 * ch + 16, :],
                          in_=s_part[:, c * B:(c + 1) * B])

    def den_matmuls(c, it, blob):
        den_ps = tiny_p.tile([1, B], F32, tag="tiny", name=f"den_ps_{it}_{c}")
        nc.tensor.matmul(den_ps[:], ones128[:], expL[:, c * B:(c + 1) * B],
                         start=True, stop=False)
        nc.tensor.matmul(den_ps[:], ones128[0:16, :],
                         expL9[:, c * B:(c + 1) * B],
                         start=False, stop=True)
        nc.scalar.copy(g_rep[0:1, c * B:(c + 1) * B], den_ps[:])
        ch = c % HC
        nc.sync.dma_start(out=blob[16 * HC + ch:16 * HC + ch + 1, :],
                          in_=g_rep[0:1, c * B:(c + 1) * B])

    def collective(blob_in, blob_out):
        nc.gpsimd.collective_compute(
            "AllReduce", ALU.add,
            replica_groups=[list(range(NCORES))],
            ins=[blob_in.opt()], outs=[blob_out.opt()],
        )

    def squash_half(it, h, blob_out):
        """v[:, half] = s_u * g for capsules [h*HC, (h+1)*HC)."""
        c0 = h * HC
        hb = slice(c0 * B, (c0 + HC) * B)
        nc.sync.dma_start(
            out=s_u[:, hb].rearrange("o (c b) -> o c b", c=HC),
            in_=blob_out[0:16 * HC, :].rearrange("(c o) b -> o c b", c=HC))
        hg = slice(10 * h, 10 * h + 10)
        src = _reshaped_ap(blob_out[16 * HC:RB, :], [[10, 128], [1, 10]])
        nc.sync.dma_start(out=den_g[:, hg], in_=src)

        nc.vector.tensor_mul(s2[:, hb], s_u[:, hb], s_u[:, hb])
        ssq_stage = dram.tile([HB], F32, tag="ssq_stage",
                              name=f"ssq_stage_{it}_{h}")
        for t in range(3):
            w = 512 if t < 2 else 256
            ssq_ps = tiny_p.tile([1, 512], F32, tag="tiny",
                                 name=f"ssq_ps_{it}_{h}_{t}")
            nc.tensor.matmul(ssq_ps[0:1, 0:w], ones16b[:],
                             s2[:, c0 * B + 512 * t:c0 * B + 512 * t + w],
                             start=True, stop=True)
            nc.scalar.copy(v_f[0:1, c0 * B + 512 * t:c0 * B + 512 * t + w],
                           ssq_ps[0:1, 0:w])
        nc.sync.dma_start(out=ssq_stage[:], in_=v_f[0:1, hb])
        src = _reshaped_ap(ssq_stage, [[10, 128], [1, 10]])
        nc.sync.dma_start(out=ssq_g[:, hg], in_=src)

        rD = gtmp_p.tile([128, 10], F32, tag="g0", name=f"g0_{it}_{h}")
        nrm = gtmp_p.tile([128, 10], F32, tag="g1", name=f"g1_{it}_{h}")
        t1 = gtmp_p.tile([128, 10], F32, tag="g2", name=f"g2_{it}_{h}")
        t2 = gtmp_p.tile([128, 10], F32, tag="g3", name=f"g3_{it}_{h}")
        g = gtmp_p.tile([128, 10], F32, tag="g4", name=f"g4_{it}_{h}")
        nc.vector.reciprocal(rD[:], den_g[:, hg])
        nc.vector.tensor_mul(t1[:], ssq_g[:, hg], rD[:])
        nc.vector.tensor_mul(nrm[:], t1[:], rD[:])
        nc.scalar.activation(t1[:], nrm[:], AF.Sqrt, bias=eps128[:])
        nc.vector.tensor_scalar_add(t2[:], nrm[:], 1.0)
        nc.vector.tensor_mul(t2[:], t2[:], t1[:])
        nc.vector.tensor_mul(t2[:], t2[:], rD[:], )
        nc.vector.reciprocal(t2[:], t2[:])
        nc.vector.tensor_mul(g[:], nrm[:], t2[:])
        nc.vector.tensor_mul(g[:], g[:], rD[:])
        nc.vector.tensor_mul(g[:], g[:], rD[:])

        g_stage = dram.tile([HB], F32, tag="g_stage", name=f"g_stage_{it}_{h}")
        dst = _reshaped_ap(g_stage, [[10, 128], [1, 10]])
        nc.sync.dma_start(out=dst, in_=g[:])
        src = _reshaped_ap(g_stage, [[0, 16], [1, HB]])
        nc.gpsimd.dma_start(out=g_rep[:, hb], in_=src)
        nc.vector.tensor_mul(v_f[:, hb], s_u[:, hb], g_rep[:, hb])
        nc.vector.tensor_copy(vb[:, hb], v_f[:, hb])

    def agreement_update(c):
        z = z_p.tile([128, NCH * B], F16, tag="z", name=f"z_{c}")
        a_ps = acc_p.tile([128, B], F32, tag="acc", name=f"a_ps_{c}")
        a9_ps = acc_p.tile([16, B], F32, tag="acc", name=f"a9_ps_{c}")
        for grp in range(3):
            j0 = 3 * grp
            u_ps = ups_p.tile([128, 3 * B], F32, tag="ups",
                              name=f"u_ps_{c}_{grp}")
            for j in range(j0, j0 + 3):
                lo = c * K + 128 * j
                nc.tensor.matmul(u_ps[:, (j - j0) * B:(j - j0 + 1) * B],
                                 w2[:, lo:lo + 128],
                                 vb[:, c * B:(c + 1) * B],
                                 start=True, stop=True)
            nc.vector.tensor_mul(z[:, j0 * B:(j0 + 3) * B],
                                 xTf[:, j0 * B:(j0 + 3) * B], u_ps[:])
            for j in range(j0, j0 + 3):
                if j < 8:
                    nc.tensor.matmul(a_ps[:], blk[:, 128 * j:128 * (j + 1)],
                                     z[:, j * B:(j + 1) * B],
                                     start=(j == 0), stop=(j == 7))
                else:
                    nc.tensor.matmul(a9_ps[:], blk[:, 1024:1040],
                                     z[:, 8 * B:9 * B], start=True, stop=True)
        nc.vector.tensor_add(L[:, c * B:(c + 1) * B],
                             L[:, c * B:(c + 1) * B], a_ps[:])
        nc.vector.tensor_add(L9[:, c * B:(c + 1) * B],
                             L9[:, c * B:(c + 1) * B], a9_ps[:])
        nc.scalar.activation(expL[:, c * B:(c + 1) * B],
                             L[:, c * B:(c + 1) * B], AF.Exp)
        nc.scalar.activation(expL9[:, c * B:(c + 1) * B],
                             L9[:, c * B:(c + 1) * B], AF.Exp)
        erep = erep_p.tile([128, NCH * B], BF16, tag="erep", name=f"erep_{c}")
        for j in range(NCH):
            s_ap = (expL[16 * j:16 * (j + 1), c * B:(c + 1) * B] if j < 8 else
                    expL9[:, c * B:(c + 1) * B])
            nc.sync.dma_start(out=erep[:, j * B:(j + 1) * B],
                              in_=_bcast_ap(s_ap, 1, CI))
        nc.vector.tensor_mul(y_all[:, c * NCH * B:(c + 1) * NCH * B],
                             xT[:], erep[:])

    # ---- pipelined schedule: half-collectives overlap the other half ----
    blobs = {}
    for r in range(NITER):
        for h in range(2):
            blobs[(r, h, "in")] = dram.tile(
                [RB, B], F32, tag=f"bi{r}{h}", name=f"blob_in_{r}_{h}")
            blobs[(r, h, "out")] = dram.tile(
                [RB, B], F32, tag=f"bo{r}{h}", name=f"blob_out_{r}_{h}")

    def work_half(r, h):
        if r > 0:
            for c in range(h * HC, (h + 1) * HC):
                agreement_update(c)
        for c in range(h * HC, (h + 1) * HC):
            s_matmuls(c, r, blobs[(r, h, "in")])
            den_matmuls(c, r, blobs[(r, h, "in")])
        collective(blobs[(r, h, "in")], blobs[(r, h, "out")])

    for r in range(NITER):
        work_half(r, 0)
        squash_half(r, 0, blobs[(r, 0, "out")])   # overlaps work_half(r,1) PE
        work_half(r, 1)
        squash_half(r, 1, blobs[(r, 1, "out")])   # overlaps work_half(r+1,0)
    nc.sync.dma_start(out=vout_d[:, :], in_=v_f[:])


def _prep_inputs(x: np.ndarray, route_weights: np.ndarray):
    """Host-side sharding + layout prep. Returns per-core input maps."""
    bf = ml_dtypes.bfloat16
    blk = _build_blockdiag()
    in_maps = []
    for k in range(NCORES):
        sl = slice(k * NLOC, (k + 1) * NLOC)
        xT = np.ascontiguousarray(
            x[:, sl, :].transpose(1, 2, 0).reshape(K, B)).astype(bf)
        w1 = np.ascontiguousarray(
            route_weights[:, sl].reshape(C, K, CO)).astype(bf)
        w1f = np.ascontiguousarray(
            route_weights[:, sl].reshape(C, K, CO)).astype(np.float32)
        w2 = np.ascontiguousarray(w1f.transpose(0, 2, 1)).astype(np.float16)
        xTf = np.ascontiguousarray(
            x[:, sl, :].transpose(1, 2, 0).reshape(K, B)).astype(np.float32)
        in_maps.append({"xT": xT, "xTf": xTf, "w1": w1, "w2": w2, "blk": blk})
    return in_maps


_NC_CACHE = {}


def _get_nc():
    if "nc" not in _NC_CACHE:
        _NC_CACHE["nc"] = build_kernel()
    return _NC_CACHE["nc"]


def _postprocess(v: np.ndarray) -> np.ndarray:
    out = v.reshape(CO, C, B).transpose(1, 2, 0)[:, :, None, None, :]
    return np.ascontiguousarray(out.astype(np.float32))


def kernel(x: np.ndarray, route_weights: np.ndarray) -> np.ndarray:
    nc = _get_nc()
    in_maps = _prep_inputs(np.asarray(x, np.float32),
                           np.asarray(route_weights, np.float32))
    res = bass_utils.run_bass_kernel_spmd(nc, in_maps,
                                          core_ids=list(range(NCORES)))
    return _postprocess(np.asarray(res.results[0]["vout"], np.float32))


def kernel_sim(x: np.ndarray, route_weights: np.ndarray) -> np.ndarray:
    """CoreSim (multi-core simulator) path for correctness debugging."""
    from concourse.bass_interp import MultiCoreSim
    nc = _get_nc()
    in_maps = _prep_inputs(np.asarray(x, np.float32),
                           np.asarray(route_weights, np.float32))
    sim = MultiCoreSim(nc, num_cores=NCORES)
    for i, core in sim.cores.items():
        for name, arr in in_maps[i].items():
            core.tensor(name)[:] = arr
    sim.simulate(check_with_hw=False)
    return _postprocess(np.asarray(sim.cores[0].tensor("vout"), np.float32))



# revision 18
# speedup vs baseline: 1.2203x; 1.2203x over previous
"""CapsuleLayer dynamic-routing kernel for 8 Trainium2 NeuronCores.

Algorithm (validated vs reference in numpy):
  priors P[c,b,n,o] = sum_i x[b,n,i] W[c,n,i,o]; logits are constant along o,
  so routing state is L[c,b,n]. Per routing iteration:
    probs = exp(L)/denom       (softmax over n; no max-subtraction: |L| < ~30)
    s[c,b,o] = sum_n probs*P = (1/denom) sum_{(n,i)} (x*exp(L)) W   <- matmul
    v = squash(s) = s_u * g,  g = rD*nrm/((1+nrm)*sqrt(nrm+eps)), nrm=ssq*rD^2
    a[c,b,n] = sum_o P*v = sum_i x * (W^T v)       <- matmul + blockdiag reduce
    L += a
  Sharding: N=1152 split 8 ways (144 route nodes/core); one 87KB AllReduce of
  s-partials + transposed softmax denominators per (iteration, capsule-half).
  Every core computes the identical full output; core 0's is returned.

Schedule (engines execute in per-engine emission order, so squash(r,h) is
emitted AFTER work(r,1-h) — otherwise DVE/PE block on the AllReduce semaphore
instead of running the other half's work):
  work(0,0) work(0,1) sq(0,0) work(1,0) sq(0,1) work(1,1) sq(1,0)
  work(2,0) sq(1,1) work(2,1) sq(2,0) sq(2,1) -> vout
A tiny "prewarm" AllReduce is issued first so the one-time ~90us collective
barrier overlaps input loads + iteration-0 compute.
"""

import struct
import sys

sys.path.insert(0, "/opt/trn_rl_repo")

import numpy as np
import ml_dtypes

import concourse.bass as bass
import concourse.bacc as bacc
import concourse.mybir as mybir
from concourse import bass_utils
from concourse.masks import make_identity
from concourse.tile import TileContext

BF16 = mybir.dt.bfloat16
F32 = mybir.dt.float32
F16 = mybir.dt.float16
I32 = mybir.dt.int32
AF = mybir.ActivationFunctionType
ALU = mybir.AluOpType

B, N, CI, C, CO = 256, 1152, 8, 10, 16
NCORES = 8
NLOC = N // NCORES          # 144 route nodes per core
K = NLOC * CI               # 1152 local contraction length (n,i)
NCH = K // 128              # 9 partition chunks of (n,i)
EPS = 1e-8
NITER = 3
CB = C * B                  # 2560
HC = C // 2                 # capsules per half
HB = HC * B                 # 1280
NS = 16 * HC * B            # s-section of the AllReduce blob (20480 f32)
ND = 128 * 2 * HC           # den^T section ([128 b, (c,bh)] = 1280 f32)
MAGIC_F = struct.unpack("f", struct.pack("I", 0x5F3759DF))[0]


def _build_blockdiag() -> np.ndarray:
    """a-reduce lhsT constants: cols 0..1023 hold 8 [128,128] blocks (chunk j
    maps (n16,i8) row q -> out partition 16j + q//8); cols 1024..1039 hold the
    9th chunk's [128,16] block (out partition q//8)."""
    blk = np.zeros((128, 8 * 128 + 16), np.float32)
    for j in range(8):
        for q in range(128):
            blk[q, 128 * j + 16 * j + q // CI] = 1.0
    for q in range(128):
        blk[q, 1024 + q // CI] = 1.0
    return blk.astype(np.float16)


def _bcast_ap(ap, dim_idx, count):
    """Insert a stride-0 (broadcast) dim into an AP at position dim_idx."""
    dims = [list(d) for d in ap.ap]
    dims.insert(dim_idx, [0, count])
    return bass.AP(tensor=ap.tensor, offset=ap.offset, ap=dims)


def _reshaped_ap(ap, dims, extra_offset=0):
    return bass.AP(tensor=ap.tensor, offset=ap.offset + extra_offset,
                   ap=[list(d) for d in dims])


def build_kernel():
    nc = bacc.Bacc("TRN2", target_bir_lowering=False, debug=False,
                   num_devices=NCORES)
    xT_d = nc.dram_tensor("xT", [K, B], BF16, kind="ExternalInput")
    xTf_d = nc.dram_tensor("xTf", [K, B], F32, kind="ExternalInput")
    w1_d = nc.dram_tensor("w1", [C, K, CO], BF16, kind="ExternalInput")
    w2_d = nc.dram_tensor("w2", [C, CO, K], F16, kind="ExternalInput")
    blk_d = nc.dram_tensor("blk", [128, 1040], F16, kind="ExternalInput")
    vout_d = nc.dram_tensor("vout", [CO, CB], F32, kind="ExternalOutput")

    with TileContext(nc) as tc:
        _emit(tc, xT_d.ap(), xTf_d.ap(), w1_d.ap(), w2_d.ap(), blk_d.ap(),
              vout_d.ap())
    nc.compile()
    return nc


def _emit(tc, xT_d, xTf_d, w1_d, w2_d, blk_d, vout_d):
    from contextlib import ExitStack
    with ExitStack() as ctx:
        _emit_body(ctx, tc, xT_d, xTf_d, w1_d, w2_d, blk_d, vout_d)


def _emit_body(ctx, tc, xT_d, xTf_d, w1_d, w2_d, blk_d, vout_d):
    nc = tc.nc
    state = ctx.enter_context(tc.tile_pool(name="state", bufs=1))
    erep_p = ctx.enter_context(tc.tile_pool(name="erep", bufs=2))
    z_p = ctx.enter_context(tc.tile_pool(name="zp", bufs=2))
    gtmp_p = ctx.enter_context(tc.tile_pool(name="gtmp", bufs=2))
    dram = ctx.enter_context(tc.tile_pool(name="dram", bufs=2, space="DRAM"))
    ups_p = ctx.enter_context(tc.tile_pool(name="ups", bufs=2, space="PSUM"))
    acc_p = ctx.enter_context(tc.tile_pool(name="acc", bufs=1, space="PSUM"))
    a9_p = ctx.enter_context(tc.tile_pool(name="a9", bufs=1, space="PSUM"))
    sps_p = ctx.enter_context(tc.tile_pool(name="sps", bufs=1, space="PSUM"))
    tiny_p = ctx.enter_context(tc.tile_pool(name="tinyps", bufs=1,
                                            space="PSUM"))

    # ---- prewarm collective: starts the one-time CC-stream barrier at t~0
    # so it overlaps input loads and iteration-0 compute.
    pre_sb = state.tile([8, 1], F32)
    pre_in = dram.tile([8], F32, tag="prei", name="pre_in")
    pre_out = dram.tile([8], F32, tag="preo", name="pre_out")
    nc.vector.memset(pre_sb[:], 1.0)
    nc.sync.dma_start(out=pre_in[:], in_=pre_sb[:])
    nc.gpsimd.collective_compute(
        "AllReduce", ALU.add, replica_groups=[list(range(NCORES))],
        ins=[pre_in.opt()], outs=[pre_out.opt()])

    # ---- persistent SBUF state ----
    xT = state.tile([128, NCH * B], BF16)        # [(n,i) chunk-part, (j, b)]
    xTf = state.tile([128, NCH * B], F32)        # fp32 copy for agreement
    w1 = state.tile([128, C * NCH * CO], BF16)   # s-matmul lhsT blocks
    w2 = state.tile([16, C * K], F16)            # U-matmul lhsT blocks
    blk = state.tile([128, 1040], F16)           # a-reduce lhsT blocks
    ident = state.tile([128, 128], F32)          # PE-transpose identity
    ones128 = state.tile([128, 1], BF16)
    ones16b = state.tile([16, 1], BF16)
    magic = state.tile([128, 2 * HC], F32)       # rsqrt bit-hack constant
    den0 = state.tile([128, 2 * HC], F32)        # denominators at r=0 (=N)
    L = state.tile([128, CB], F32)               # logits, partition = local n
    L9 = state.tile([16, CB], F32)               # local n in [128,144)
    expL = state.tile([128, CB], BF16)
    expL9 = state.tile([16, CB], BF16)
    y_all = state.tile([128, C * NCH * B], BF16)  # y = x*expL per c
    s_part = state.tile([16, CB], F32)           # local s partials (pre-AR)
    s_u = state.tile([16, CB], F32)              # AllReduced s_unnorm [o,(c,b)]
    s2 = state.tile([16, CB], BF16)              # s_u^2 (ssq matmul lhsT)
    g_rep = state.tile([16, CB], F32)            # squash scale, bcast over o
    v_f = state.tile([16, CB], F32)
    vb = state.tile([16, CB], F16)

    # ---- load inputs / init state ----
    for j in range(NCH):
        nc.sync.dma_start(out=xT[:, j * B:(j + 1) * B],
                          in_=xT_d[j * 128:(j + 1) * 128, :])
        nc.sync.dma_start(out=xTf[:, j * B:(j + 1) * B],
                          in_=xTf_d[j * 128:(j + 1) * 128, :])
    for c in range(C):
        src = w1_d[c].rearrange("(j p) o -> p j o", j=NCH)
        dst = w1[:, c * NCH * CO:(c + 1) * NCH * CO].rearrange(
            "p (j o) -> p j o", j=NCH)
        nc.sync.dma_start(out=dst, in_=src)
    nc.sync.dma_start(out=w2[:].rearrange("p (c k) -> p c k", c=C),
                      in_=w2_d.rearrange("c o k -> o c k"))
    nc.sync.dma_start(out=blk[:], in_=blk_d[:, :])
    make_identity(nc, ident)
    nc.vector.memset(ones128[:], 1.0)
    nc.vector.memset(ones16b[:], 1.0)
    nc.vector.memset(magic[:], MAGIC_F)
    nc.vector.memset(den0[:], float(N))
    nc.vector.memset(L[:], 0.0)
    nc.vector.memset(L9[:], 0.0)
    nc.vector.memset(expL[:], 1.0)   # exp(0)
    nc.vector.memset(expL9[:], 1.0)

    # AllReduce blobs: flat f32 [NS] (r=0) or [NS+ND] (r>0). s-section rows
    # (c',o) x b; den^T section [128 b-part, (c',bh)] (so the squash needs no
    # post-AllReduce transposes).
    blobs = {}
    for r in range(NITER):
        sz = NS + (ND if r > 0 else 0)
        for h in range(2):
            blobs[(r, h, "in")] = dram.tile(
                [sz], F32, tag=f"bi{r}{h}", name=f"blob_in_{r}_{h}")
            blobs[(r, h, "out")] = dram.tile(
                [sz], F32, tag=f"bo{r}{h}", name=f"blob_out_{r}_{h}")

    def collective(r, h):
        nc.gpsimd.collective_compute(
            "AllReduce", ALU.add, replica_groups=[list(range(NCORES))],
            ins=[blobs[(r, h, "in")].opt()], outs=[blobs[(r, h, "out")].opt()])

    def s_mm(c, r, blob):
        """s-partial matmuls for one capsule (r>0), PSUM -> blob directly."""
        s_ps = sps_p.tile([16, B], F32, tag="sps", name=f"s_ps_{r}_{c}")
        for j in range(NCH):
            lo = (c * NCH + j) * CO
            nc.tensor.matmul(s_ps[:], w1[:, lo:lo + CO],
                             y_all[:, (c * NCH + j) * B:(c * NCH + j + 1) * B],
                             start=(j == 0), stop=(j == NCH - 1))
        ch = c % HC
        nc.scalar.copy(s_part[:, c * B:(c + 1) * B], s_ps[:])
        nc.sync.dma_start(
            out=_reshaped_ap(blob, [[B, 16], [1, B]], extra_offset=16 * ch * B),
            in_=s_part[:, c * B:(c + 1) * B])

    def s_mm_r0(h, blob):
        """Iteration 0: y == xT for every capsule (all of this hides under
        the one-time collective barrier, so no batching needed)."""
        for cl in range(HC):
            c = h * HC + cl
            s_ps = sps_p.tile([16, B], F32, tag="sps", name=f"s0_ps_{c}")
            for j in range(NCH):
                lo = (c * NCH + j) * CO
                nc.tensor.matmul(s_ps[:], w1[:, lo:lo + CO],
                                 xT[:, j * B:(j + 1) * B],
                                 start=(j == 0), stop=(j == NCH - 1))
            nc.scalar.copy(s_part[:, c * B:(c + 1) * B], s_ps[:])
            nc.sync.dma_start(
                out=_reshaped_ap(blob, [[B, 16], [1, B]],
                                 extra_offset=16 * cl * B),
                in_=s_part[:, c * B:(c + 1) * B])

    def den_mm(h, r, blob):
        """Transposed local softmax denominators: den_t[b, (c',bh)] =
        sum_n expL[n, (c,128bh+b)] via 1-col matmuls with expL as lhsT."""
        den_t = tiny_p.tile([128, 2 * HC], F32, tag="tps", name=f"dent_{r}_{h}")
        for cl in range(HC):
            c = h * HC + cl
            for bh in range(2):
                t = 2 * cl + bh
                sl = slice(c * B + bh * 128, c * B + bh * 128 + 128)
                nc.tensor.matmul(den_t[:, t:t + 1], expL[:, sl], ones128[:],
                                 start=True, stop=False)
                nc.tensor.matmul(den_t[:, t:t + 1], expL9[:, sl], ones16b[:],
                                 start=False, stop=True)
        den_sb = gtmp_p.tile([128, 2 * HC], F32, tag="dsb",
                             name=f"dsb_{r}_{h}")
        nc.vector.tensor_copy(den_sb[:], den_t[:])
        nc.sync.dma_start(
            out=_reshaped_ap(blob, [[2 * HC, 128], [1, 2 * HC]],
                             extra_offset=NS),
            in_=den_sb[:])

    def u_emit(c, r, zt, uu):
        """u = W^T v for capsule c (9 matmuls, 3 PSUM groups), then z =
        xTf * u on DVE into slot uu of pair tile zt."""
        for g in range(3):
            u_ps = ups_p.tile([128, 3 * B], F32, tag="ups",
                              name=f"u_ps_{r}_{c}_{g}")
            for j in range(3 * g, 3 * g + 3):
                lo = c * K + 128 * j
                nc.tensor.matmul(u_ps[:, (j - 3 * g) * B:(j - 3 * g + 1) * B],
                                 w2[:, lo:lo + 128], vb[:, c * B:(c + 1) * B],
                                 start=True, stop=True)
            zlo = uu * NCH * B + 3 * g * B
            nc.vector.tensor_mul(zt[:, zlo:zlo + 3 * B],
                                 xTf[:, 3 * g * B:(3 * g + 3) * B], u_ps[:])

    def finish_caps(cs, r, a2_ps, a9_ps):
        """L += a, expL = exp(L), erep broadcast, y = xT*expL for capsules cs
        (cs contiguous; a2/a9 psum cols laid out (uu, b))."""
        nn = len(cs) * B
        c0 = cs[0]
        nc.vector.tensor_add(L[:, c0 * B:c0 * B + nn],
                             L[:, c0 * B:c0 * B + nn], a2_ps[:])
        nc.vector.tensor_add(L9[:, c0 * B:c0 * B + nn],
                             L9[:, c0 * B:c0 * B + nn], a9_ps[:])
        nc.scalar.activation(expL[:, c0 * B:c0 * B + nn],
                             L[:, c0 * B:c0 * B + nn], AF.Exp)
        nc.scalar.activation(expL9[:, c0 * B:c0 * B + nn],
                             L9[:, c0 * B:c0 * B + nn], AF.Exp)
        for c in cs:
            erep = erep_p.tile([128, NCH * B], BF16, tag="erep",
                               name=f"erep_{r}_{c}")
            for j in range(NCH):
                s_ap = (expL[16 * j:16 * (j + 1), c * B:(c + 1) * B] if j < 8
                        else expL9[:, c * B:(c + 1) * B])
                eng = nc.sync if j < 5 else nc.gpsimd
                eng.dma_start(out=erep[:, j * B:(j + 1) * B],
                              in_=_bcast_ap(s_ap, 1, CI))
            nc.vector.tensor_mul(y_all[:, c * NCH * B:(c + 1) * NCH * B],
                                 xT[:], erep[:])

    def a_reduce(cs, r, zt):
        """Blockdiag i-reduction for 1 or 2 capsules in one 256/512-col
        stream per chunk, then the L/expL/erep/y chain."""
        nu = len(cs)
        a2_ps = acc_p.tile([128, nu * B], F32, tag="acc",
                           name=f"a2_{r}_{cs[0]}")
        a9_ps = a9_p.tile([16, nu * B], F32, tag="a9", name=f"a9_{r}_{cs[0]}")
        for j in range(NCH):
            rhs = _reshaped_ap(zt, [list(zt.ap[0]), [NCH * B, nu], [1, B]],
                               extra_offset=j * B)
            if j < 8:
                nc.tensor.matmul(a2_ps[:], blk[:, 128 * j:128 * (j + 1)], rhs,
                                 start=(j == 0), stop=(j == 7))
            else:
                nc.tensor.matmul(a9_ps[:], blk[:, 1024:1040], rhs,
                                 start=True, stop=True)
        finish_caps(cs, r, a2_ps, a9_ps)

    def work_half(r, h):
        blob = blobs[(r, h, "in")]
        if r == 0:
            s_mm_r0(h, blob)
            collective(r, h)
            return
        cs = list(range(h * HC, (h + 1) * HC))
        z2a = z_p.tile([128, 2 * NCH * B], F16, tag="z2", name=f"z2a_{r}_{h}")
        z2b = z_p.tile([128, 2 * NCH * B], F16, tag="z2", name=f"z2b_{r}_{h}")
        zs = z_p.tile([128, NCH * B], F16, tag="zs", bufs=1,
                      name=f"zs_{r}_{h}")
        # PE-continuity interleave: u's run ahead so a's never wait on DVE z.
        u_emit(cs[0], r, z2a, 0)
        u_emit(cs[1], r, z2a, 1)
        u_emit(cs[2], r, z2b, 0)
        a_reduce(cs[0:2], r, z2a)
        u_emit(cs[3], r, z2b, 1)
        u_emit(cs[4], r, zs, 0)
        a_reduce(cs[2:4], r, z2b)
        a_reduce(cs[4:5], r, zs)
        for c in cs:
            s_mm(c, r, blob)
        den_mm(h, r, blob)
        collective(r, h)

    def squash(r, h):
        """Post-AllReduce: v = s_u * g. All squash state lives in
        [128 b-part, (c',bh)] layout so no transposes are needed except one
        PE transpose of g back to row form for the o-broadcast."""
        blob = blobs[(r, h, "out")]
        hb = slice(h * HB, (h + 1) * HB)
        nc.sync.dma_start(
            out=s_u[:, hb].rearrange("o (c b) -> o c b", c=HC),
            in_=_reshaped_ap(blob, [[B, HC * 16], [1, B]]).rearrange(
                "(c o) b -> o c b", c=HC))
        if r > 0:
            dT = gtmp_p.tile([128, 2 * HC], F32, tag="dT", name=f"dT_{r}_{h}")
            nc.sync.dma_start(
                out=dT[:],
                in_=_reshaped_ap(blob, [[2 * HC, 128], [1, 2 * HC]],
                                 extra_offset=NS))
        else:
            dT = den0
        nc.vector.tensor_mul(s2[:, hb], s_u[:, hb], s_u[:, hb])
        ssq_t = tiny_p.tile([128, 2 * HC], F32, tag="tps",
                            name=f"ssqt_{r}_{h}")
        for t in range(2 * HC):
            lo = h * HB + t * 128
            nc.tensor.matmul(ssq_t[:, t:t + 1], s2[:, lo:lo + 128],
                             ones16b[:], start=True, stop=True)
        # g = rD * nrm / ((1+nrm) * sqrt(nrm+eps)); rsqrt via DVE bit-hack
        # + 2 Newton steps (no ACT table swap away from exp).
        W = 2 * HC
        rD = gtmp_p.tile([128, W], F32, tag="g0", name=f"g0_{r}_{h}")
        nrm = gtmp_p.tile([128, W], F32, tag="g1", name=f"g1_{r}_{h}")
        xe = gtmp_p.tile([128, W], F32, tag="g2", name=f"g2_{r}_{h}")
        q = gtmp_p.tile([128, W], F32, tag="g3", name=f"g3_{r}_{h}")
        t1 = gtmp_p.tile([128, W], F32, tag="g4", name=f"g4_{r}_{h}")
        g = gtmp_p.tile([128, W], F32, tag="g5", name=f"g5_{r}_{h}")
        nc.vector.reciprocal(rD[:], dT[:])
        nc.vector.tensor_mul(nrm[:], ssq_t[:], rD[:])
        nc.vector.tensor_mul(nrm[:], nrm[:], rD[:])
        nc.vector.tensor_scalar_add(xe[:], nrm[:], EPS)
        nc.vector.tensor_scalar(out=q.bitcast(I32), in0=xe.bitcast(I32),
                                scalar1=1, scalar2=None,
                                op0=ALU.logical_shift_right)
        nc.vector.tensor_sub(q.bitcast(I32), magic.bitcast(I32),
                             q.bitcast(I32))
        for _ in range(2):
            nc.vector.tensor_mul(t1[:], q[:], q[:])
            nc.vector.tensor_mul(t1[:], t1[:], xe[:])
            nc.vector.tensor_scalar(out=t1[:], in0=t1[:], scalar1=-0.5,
                                    scalar2=1.5, op0=ALU.mult, op1=ALU.add)
            nc.vector.tensor_mul(q[:], q[:], t1[:])
        nc.vector.tensor_scalar_add(t1[:], nrm[:], 1.0)
        nc.vector.reciprocal(t1[:], t1[:])
        nc.vector.tensor_mul(g[:], nrm[:], q[:])
        nc.vector.tensor_mul(g[:], g[:], t1[:])
        nc.vector.tensor_mul(g[:], g[:], rD[:])
        # g [128,(c',bh)] -> row layout, broadcast over the 16 o-partitions.
        gT_ps = a9_p.tile([2 * HC, 128], F32, tag="a9", name=f"gT_{r}_{h}")
        nc.tensor.transpose(gT_ps[:], g[:], ident[:])
        gT = gtmp_p.tile([2 * HC, 128], F32, tag="gTs", name=f"gTs_{r}_{h}")
        nc.vector.tensor_copy(gT[:], gT_ps[:])
        for t in range(2 * HC):
            nc.scalar.dma_start(
                out=g_rep[:, h * HB + t * 128:h * HB + (t + 1) * 128],
                in_=_bcast_ap(gT[t:t + 1, :], 1, 16))
        if r < NITER - 1:
            nc.vector.tensor_mul(vb[:, hb], s_u[:, hb], g_rep[:, hb])
        else:
            nc.vector.tensor_mul(v_f[:, hb], s_u[:, hb], g_rep[:, hb])

    # ---- schedule: squash(r,h) is emitted after work(r,1-h) so no engine
    # parks on an AllReduce semaphore while independent work remains.
    work_half(0, 0)
    work_half(0, 1)
    squash(0, 0)
    work_half(1, 0)
    squash(0, 1)
    work_half(1, 1)
    squash(1, 0)
    work_half(2, 0)
    squash(1, 1)
    work_half(2, 1)
    squash(2, 0)
    squash(2, 1)
    nc.sync.dma_start(out=vout_d[:, :], in_=v_f[:])


def _prep_inputs(x: np.ndarray, route_weights: np.ndarray):
    """Host-side sharding + layout prep. Returns per-core input maps."""
    bf = ml_dtypes.bfloat16
    blk = _build_blockdiag()
    in_maps = []
    for k in range(NCORES):
        sl = slice(k * NLOC, (k + 1) * NLOC)
        xT = np.ascontiguousarray(
            x[:, sl, :].transpose(1, 2, 0).reshape(K, B)).astype(bf)
        w1 = np.ascontiguousarray(
            route_weights[:, sl].reshape(C, K, CO)).astype(bf)
        w1f = np.ascontiguousarray(
            route_weights[:, sl].reshape(C, K, CO)).astype(np.float32)
        w2 = np.ascontiguousarray(w1f.transpose(0, 2, 1)).astype(np.float16)
        xTf = np.ascontiguousarray(
            x[:, sl, :].transpose(1, 2, 0).reshape(K, B)).astype(np.float32)
        in_maps.append({"xT": xT, "xTf": xTf, "w1": w1, "w2": w2, "blk": blk})
    return in_maps


_NC_CACHE = {}


def _get_nc():
    if "nc" not in _NC_CACHE:
        _NC_CACHE["nc"] = build_kernel()
    return _NC_CACHE["nc"]


def _postprocess(v: np.ndarray) -> np.ndarray:
    out = v.reshape(CO, C, B).transpose(1, 2, 0)[:, :, None, None, :]
    return np.ascontiguousarray(out.astype(np.float32))


def kernel(x: np.ndarray, route_weights: np.ndarray) -> np.ndarray:
    nc = _get_nc()
    in_maps = _prep_inputs(np.asarray(x, np.float32),
                           np.asarray(route_weights, np.float32))
    res = bass_utils.run_bass_kernel_spmd(nc, in_maps,
                                          core_ids=list(range(NCORES)))
    return _postprocess(np.asarray(res.results[0]["vout"], np.float32))


def kernel_sim(x: np.ndarray, route_weights: np.ndarray) -> np.ndarray:
    """CoreSim (multi-core simulator) path for correctness debugging."""
    from concourse.bass_interp import MultiCoreSim
    nc = _get_nc()
    in_maps = _prep_inputs(np.asarray(x, np.float32),
                           np.asarray(route_weights, np.float32))
    sim = MultiCoreSim(nc, num_cores=NCORES)
    for i, core in sim.cores.items():
        for name, arr in in_maps[i].items():
            core.tensor(name)[:] = arr
    sim.simulate(check_with_hw=False)
    return _postprocess(np.asarray(sim.cores[0].tensor("vout"), np.float32))
